# revision 21
# baseline (speedup 1.0000x reference)
"""Trainium2 Bass kernel for NeuroplasticLlama block-sparse adapter (moe_routing).

Contract: kernel(**inputs) takes FULL unsharded inputs (as produced by
setup_inputs) and returns the FULL [4, 4096, 4096] float32 output.

Strategy (data/sequence parallel over 8 cores, 2048 tokens each):
  - Each core's 2048 contiguous tokens belong to exactly one batch, so the
    task embedding contributes only per-core constant bias vectors
    (te @ A folded into the z bias, te @ Wp folded into the coords bias)
    -- h = x + te is never materialized.
  - Routing is rank-3: scores s[t,n] = coords[t]·mu_n - |mu_n|^2/2 with
    coords = x @ Wp + (te @ Wp + bp).  coords is a K=4096 fp8-DoubleRow
    matmul with M=3; scores are then tiny K=4 matmuls producing s token-major
    [t, n] directly (no score transposes).  The per-token shift -|coords|^2/2
    is dropped (softmax over top-k and the top-k set are shift invariant).
  - top-3 selection via threshold = 3rd max (3 rounds of max + mask-out),
    gates g[t,n] = exp(s - max) * (s >= thr3) / sum(...)  (DVE chain).
  - z[t,:] (all 512 block-rank pairs) = x @ A_all, dense fp8 DoubleRow.
    zg = (z/8) * expand4(g) in fp8; delta = (8*block-diag(Bm)) matmul fp8;
    y = x + delta accumulated in bf16 SBUF and stored.
  - I/O is bf16 (host converts): x arrives pre-packed [128, 65536] bf16 in
    the exact SBUF tile order (fully contiguous DMA), y leaves the same way
    and is unpacked + upcast on host.  rel-err budget 2e-2 >> bf16 noise.
"""

import sys

if "/opt/trn_rl_repo" not in sys.path:
    sys.path.insert(0, "/opt/trn_rl_repo")

import numpy as np
import ml_dtypes

H = 4096
NB = 128
BLK = 32
R = 4
B = 4
S = 4096
NCORES = 8
TPC = (B * S) // NCORES  # tokens per core = 2048
T = 512                  # tokens per macrotile
NMT = TPC // T           # 4 macrotiles per core
NKT = H // 128           # 32 k-tiles over the hidden dim
BIG = 1.0e30
ZSC = 0.125              # z is scaled by 1/8 before fp8, Bm by 8

TRACE = False            # set by test.py for profiling runs
TRACE_DIR = None
LAST_RESULT = None       # BassKernelResults of the last run

_COMPILED = None


def _build():
    import concourse.bacc as bacc
    import concourse.tile as tile
    from concourse import mybir, masks

    f32 = mybir.dt.float32
    bf16 = mybir.dt.bfloat16
    f8 = mybir.dt.float8e4
    AF = mybir.ActivationFunctionType
    AL = mybir.AluOpType
    AX = mybir.AxisListType
    DR = mybir.MatmulPerfMode.DoubleRow

    nc = bacc.Bacc("TRN2", target_bir_lowering=False, debug=False,
                   num_devices=NCORES)

    xt_d = nc.dram_tensor("xt", [128, NMT * 8 * 2048], bf16,
                          kind="ExternalInput")
    xb_d = nc.dram_tensor("xb8", [128, NMT * 8 * 2048], f8,
                          kind="ExternalInput")
    az_d = nc.dram_tensor("az", [128, 4 * NKT * 128], f8, kind="ExternalInput")
    wp_d = nc.dram_tensor("wp", [128, NKT * 16], f8, kind="ExternalInput")
    bpk_d = nc.dram_tensor("bpk", [128, NKT * 128], f8, kind="ExternalInput")
    e_d = nc.dram_tensor("e", [128, 512], bf16, kind="ExternalInput")
    cen_d = nc.dram_tensor("cen", [4, 128], bf16, kind="ExternalInput")
    bias_d = nc.dram_tensor("bias", [128, 5], f32, kind="ExternalInput")
    yt_d = nc.dram_tensor("yt", [128, NMT * 8 * 2048], bf16,
                          kind="ExternalOutput")

    xt_ap = xt_d.ap()
    xb_ap = xb_d.ap()
    yt_ap = yt_d.ap()

    with tile.TileContext(nc) as tc:
        from contextlib import ExitStack
        with ExitStack() as ctx:
            cpool = ctx.enter_context(tc.tile_pool(name="consts", bufs=1))
            xpool = ctx.enter_context(tc.tile_pool(name="xg", bufs=16))
            xbpool = ctx.enter_context(tc.tile_pool(name="xb", bufs=16))
            zpool = ctx.enter_context(tc.tile_pool(name="zb", bufs=6))
            gpool = ctx.enter_context(tc.tile_pool(name="gate", bufs=3))
            spool = ctx.enter_context(tc.tile_pool(name="scal", bufs=4))
            pp = ctx.enter_context(tc.tile_pool(name="ps", bufs=2, space="PSUM"))

            # ---- persistent constants ----
            wp8 = cpool.tile([128, NKT * 16], f8, name="wp8", tag="wp8")
            nc.gpsimd.dma_start(wp8[:], wp_d.ap()[:])
            cen = cpool.tile([4, 128], bf16, name="cen", tag="cen")
            nc.gpsimd.dma_start(cen[:], cen_d.ap()[:])
            bias = cpool.tile([128, 5], f32, name="bias", tag="bias")
            nc.gpsimd.dma_start(bias[:], bias_d.ap()[:])
            az = []
            for q in range(4):
                t_az = cpool.tile([128, NKT * 128], f8, name=f"az{q}",
                                  tag=f"az{q}")
                nc.gpsimd.dma_start(
                    t_az[:], az_d.ap()[:, q * NKT * 128:(q + 1) * NKT * 128])
                az.append(t_az)
            esb = cpool.tile([128, 512], bf16, name="esb", tag="esb")
            nc.gpsimd.dma_start(esb[:], e_d.ap()[:])
            bpk = cpool.tile([128, NKT * 128], f8, name="bpk", tag="bpk")
            nc.gpsimd.dma_start(bpk[:], bpk_d.ap()[:])
            identf = cpool.tile([128, 128], f32, name="identf", tag="identf")
            masks.make_identity(nc, identf[:])

            NTS = T // 128  # token sub-tiles per macrotile

            def issue_loads(mt):
                base = mt * 8 * 2048
                xbs, xgs = [], []
                for g in range(8):
                    xb = xbpool.tile([128, 4 * T], f8, name="xb", tag="xb")
                    eng = nc.scalar if (mt == 0 and g % 2 == 1) else nc.sync
                    eng.dma_start(xb[:],
                                  xb_ap[:, base + g * 2048:base + (g + 1) * 2048])
                    xbs.append(xb)
                for g in range(8):
                    xg = xpool.tile([128, 4 * T], bf16, name="xg", tag="xg")
                    eng = nc.scalar if (mt == 0 and g % 2 == 1) else nc.sync
                    eng.dma_start(xg[:],
                                  xt_ap[:, base + g * 2048:base + (g + 1) * 2048])
                    xgs.append(xg)
                return xbs, xgs

            def gates_frontend(xbs):
                # coords chain (fp8 DR, M=3 padded to 16)
                cp = pp.tile([16, T], f32, space="PSUM", name="cp", tag="cp",
                             bufs=1)
                for k2 in range(NKT // 2):
                    g, hl = k2 // 2, (k2 % 2) * 2
                    nc.tensor.matmul(
                        cp[:],
                        wp8[:, k2 * 32:(k2 + 1) * 32]
                        .rearrange("p (two m) -> p two m", two=2),
                        xbs[g][:, hl * T:(hl + 2) * T]
                        .rearrange("p (two t) -> p two t", two=2),
                        start=(k2 == 0), stop=(k2 == NKT // 2 - 1),
                        perf_mode=DR,
                    )
                csb = gpool.tile([4, T], bf16, name="csb", tag="csb", bufs=2)
                nc.gpsimd.memset(csb[:], 1.0)
                nc.scalar.activation(csb[0:3, :], cp[0:3, :], AF.Identity,
                                     bias=bias[0:3, 4:5], scale=1.0)
                # scores (token-major) + DVE gate chain
                sp = pp.tile([128, 4 * 128], f32, space="PSUM", name="sp",
                             tag="sp", bufs=1)
                for ts in range(NTS):
                    nc.tensor.matmul(sp[:, ts * 128:(ts + 1) * 128],
                                     csb[:, ts * 128:(ts + 1) * 128],
                                     cen[:], start=True, stop=True)
                ggs = []
                for ts in range(NTS):
                    ssl = sp[:, ts * 128:(ts + 1) * 128]
                    m8 = spool.tile([128, 8], f32, name="m8", tag="m8")
                    nc.vector.max(m8[:], ssl)
                    nr1 = spool.tile([128, 1], f32, name="nr1", tag="nr1")
                    nc.vector.tensor_scalar_mul(nr1[:], m8[:, 0:1], -1.0)
                    ex = gpool.tile([128, 128], f32, name="ex", tag="ex")
                    nc.scalar.activation(ex[:], ssl, AF.Exp, bias=nr1[:],
                                         scale=1.0)
                    em = gpool.tile([128, 128], f32, name="em", tag="em")
                    zs = spool.tile([128, 1], f32, name="zs", tag="zs")
                    nc.vector.scalar_tensor_tensor(em[:], ssl, m8[:, 2:3],
                                                   ex[:], AL.is_ge, AL.mult,
                                                   accum_out=zs[:])
                    rz = spool.tile([128, 1], f32, name="rz", tag="rz")
                    nc.vector.reciprocal(rz[:], zs[:])
                    gg = gpool.tile([128, 128], f32, name="gg", tag="gg",
                                    bufs=NTS + 1)
                    nc.scalar.mul(gg[:], em[:], rz[:])
                    ggs.append(gg)
                return ggs

            xbs, xgs = issue_loads(0)
            ggs = gates_frontend(xbs)

            for mt in range(NMT):
                base = mt * 8 * 2048
                # ---- z chunks (fp8 DoubleRow) ----
                zbs = []
                for q in range(4):
                    zp = pp.tile([128, T], f32, space="PSUM", name="zp",
                                 tag="zp")
                    for k2 in range(NKT // 2):
                        g, hl = k2 // 2, (k2 % 2) * 2
                        nc.tensor.matmul(
                            zp[:],
                            az[q][:, k2 * 256:(k2 + 1) * 256]
                            .rearrange("p (two m) -> p two m", two=2),
                            xbs[g][:, hl * T:(hl + 2) * T]
                            .rearrange("p (two t) -> p two t", two=2),
                            start=(k2 == 0), stop=(k2 == NKT // 2 - 1),
                            perf_mode=DR,
                        )
                    zb = zpool.tile([128, T], bf16, name="zb", tag="zb")
                    nc.scalar.activation(zb[:], zp[:], AF.Identity,
                                         bias=bias[:, q:q + 1], scale=ZSC)
                    zbs.append(zb)

                # ---- next MT frontend (pipelined: DVE chain runs ahead of
                # this MT's drains in the in-order engine streams) ----
                if mt + 1 < NMT:
                    nxbs, nxgs = issue_loads(mt + 1)
                    nggs = gates_frontend(nxbs)

                # ---- gates: transpose (PE, bf16) + expand matmuls ----
                gt_sb = gpool.tile([128, T], bf16, name="gt_sb", tag="gt_sb",
                                   bufs=2)
                g_ps = pp.tile([128, 4 * 128], f32, space="PSUM", name="g_ps",
                               tag="sp", bufs=1)
                for ts in range(NTS):
                    nc.tensor.transpose(g_ps[:, ts * 128:(ts + 1) * 128],
                                        ggs[ts][:], identf[:])
                nc.scalar.copy(gt_sb[:], g_ps[:])

                # ---- apply gates, delta, add into x, store ----
                for q in range(4):
                    gx = pp.tile([128, T], f32, space="PSUM", name="gx",
                                 tag="zp", bufs=2)
                    nc.tensor.matmul(gx[:],
                                     esb[:, q * 128:(q + 1) * 128],
                                     gt_sb[:],
                                     start=True, stop=True)
                    zg = zpool.tile([128, T], f8, name="zg", tag="zg")
                    nc.vector.tensor_mul(zg[:], zbs[q][:], gx[:])
                    for pair in range(4):
                        hc0 = q * 8 + pair * 2
                        g = hc0 // 4
                        dp = pp.tile([128, 2 * T], f32, space="PSUM", name="dp",
                                     tag="dp", bufs=2)
                        for h in range(2):
                            hc = hc0 + h
                            nc.tensor.matmul(dp[:, h * T:(h + 1) * T],
                                             bpk[:, hc * 128:(hc + 1) * 128],
                                             zg[:],
                                             start=True, stop=True)
                        xsl = xgs[g][:, (hc0 % 4) * T:(hc0 % 4 + 2) * T]
                        if (q + pair) % 5 == 4:
                            nc.vector.tensor_add(xsl, xsl, dp[:])
                        else:
                            dsb = zpool.tile([128, 2 * T], bf16, name="dsb",
                                             tag="dsb", bufs=3)
                            nc.scalar.copy(dsb[:], dp[:])
                            nc.vector.tensor_add(xsl, xsl, dsb[:])
                        if hc0 % 4 == 2:
                            seng = nc.sync if (mt == NMT - 1 and g % 2 == 1) \
                                else nc.gpsimd
                            seng.dma_start(
                                yt_ap[:, base + g * 2048:base + (g + 1) * 2048],
                                xgs[g][:])
                if mt + 1 < NMT:
                    xbs, xgs, ggs = nxbs, nxgs, nggs

    nc.compile()
    return nc


def _prep_consts(task_emb, task_ids, Wp, bp, centers, A, Bm, adapter_scale):
    scale = float(np.asarray(adapter_scale))
    A_all = np.ascontiguousarray(
        A.transpose(1, 0, 2).reshape(H, NB * R).astype(np.float32))

    # az: [p, q, hc, m] = A_all[hc*128+p, q*128+m], fp8 e4m3 (DoubleRow pairs
    # of consecutive hc become the [K,2,M] interleave)
    az = (A_all.reshape(NKT, 128, 4, 128).transpose(1, 2, 0, 3)
          .reshape(128, 4 * NKT * 128).astype(ml_dtypes.float8_e4m3))
    az = np.ascontiguousarray(az)

    # wp8: [p, k2, two, c] = Wp[(2*k2+two)*128+p, c] (c padded 3->16:
    # fp8 DoubleRow LDWEIGHTS requires the pair stride to be 16B-aligned), fp8
    wpp = np.zeros((H, 16), np.float32)
    wpp[:, 0:3] = Wp.astype(np.float32)
    wp8 = (wpp.reshape(NKT // 2, 2, 128, 16)
           .transpose(2, 0, 1, 3).reshape(128, NKT * 16)
           .astype(ml_dtypes.float8_e4m3))
    wp8 = np.ascontiguousarray(wp8)

    # block-diag up-projection (x8 to keep fp8 in normal range; z is /8)
    bpk = np.zeros((128, NKT * 128), np.float32)
    for hc in range(NKT):
        for mblk in range(4):
            n = hc * 4 + mblk
            for r in range(R):
                row = (hc % 8) * 16 + mblk * 4 + r
                bpk[row, hc * 128 + mblk * 32: hc * 128 + mblk * 32 + 32] = \
                    Bm[n, r, :] * scale * 8.0
    bpk = bpk.astype(ml_dtypes.float8_e4m3)

    e_np = (np.arange(128)[:, None] == (np.arange(512)[None, :] // 4)) \
        .astype(ml_dtypes.bfloat16)

    # cen_aug: rows 0-2 = centers.T, row 3 = -|mu|^2/2
    cen = np.zeros((4, 128), np.float32)
    cen[0:3] = centers.T
    cen[3] = -0.5 * (centers ** 2).sum(-1)
    cen = np.ascontiguousarray(cen.astype(ml_dtypes.bfloat16))

    biases = []
    for c in range(NCORES):
        te = task_emb[int(np.asarray(task_ids)[c // 2])].astype(np.float32)
        b5 = np.zeros((128, 5), np.float32)
        zoff = (te @ A_all) * ZSC                                # [512]
        for q in range(4):
            b5[:, q] = zoff[q * 128:(q + 1) * 128]
        b5[0:3, 4] = te @ Wp + bp                                # coords bias
        biases.append(np.ascontiguousarray(b5))
    return az, wp8, bpk, e_np, cen, biases


def _pack_x(xc):
    # [TPC, H] f32 -> [128, NMT*8*2048] in kernel tile order (bf16 + fp8)
    t = np.ascontiguousarray(xc.reshape(NMT, T, 8, 4, 128)
                             .transpose(4, 0, 2, 3, 1)
                             .reshape(128, NMT * 8 * 2048))
    return t.astype(ml_dtypes.bfloat16), t.astype(ml_dtypes.float8_e4m3)


def _unpack_y(yt):
    # [128, NMT*8*2048] bf16 -> [TPC, H] f32
    t = yt.reshape(128, NMT, 8, 4, T).transpose(1, 4, 2, 3, 0)
    return t.reshape(TPC, H).astype(np.float32)


def kernel(x, task_ids, task_emb, Wp, bp, centers, A, Bm, adapter_scale):
    global _COMPILED, LAST_RESULT
    from concourse import bass_utils

    x = np.asarray(x, dtype=np.float32)
    task_ids = np.asarray(task_ids)
    task_emb = np.asarray(task_emb, dtype=np.float32)
    Wp = np.asarray(Wp, dtype=np.float32)
    bp = np.asarray(bp, dtype=np.float32)
    centers = np.asarray(centers, dtype=np.float32)
    A = np.asarray(A, dtype=np.float32)
    Bm = np.asarray(Bm, dtype=np.float32)

    if _COMPILED is None:
        _COMPILED = _build()
    nc = _COMPILED

    az, wp8, bpk, e_np, cen, biases = _prep_consts(
        task_emb, task_ids, Wp, bp, centers, A, Bm, adapter_scale)

    xf = x.reshape(B * S, H)
    in_maps = []
    for c in range(NCORES):
        xtc, xbc = _pack_x(xf[c * TPC:(c + 1) * TPC])
        in_maps.append({"xt": xtc, "xb8": xbc, "az": az, "wp": wp8,
                        "bpk": bpk, "e": e_np, "cen": cen,
                        "bias": biases[c]})

    kwargs = {}
    if TRACE:
        kwargs = dict(trace=True, tmpdir=TRACE_DIR)
    res = bass_utils.run_bass_kernel_spmd(
        nc, in_maps, core_ids=list(range(NCORES)), **kwargs)
    LAST_RESULT = res

    out = np.empty((B * S, H), np.float32)
    for c in range(NCORES):
        out[c * TPC:(c + 1) * TPC] = _unpack_y(res.results[c]["yt"])
    return out.reshape(B, S, H)


# revision 22
# speedup vs baseline: 1.1149x; 1.1149x over previous
"""Trainium2 Bass kernel for NeuroplasticLlama block-sparse adapter (moe_routing).

Contract: kernel(**inputs) takes FULL unsharded inputs (as produced by
setup_inputs) and returns the FULL [4, 4096, 4096] float32 output.

Strategy (data/sequence parallel over 8 cores, 2048 tokens each):
  - Each core's 2048 contiguous tokens belong to exactly one batch, so the
    task embedding contributes only per-core constant bias vectors
    (te @ A folded into the z bias, te @ Wp folded into the coords bias)
    -- h = x + te is never materialized.
  - Routing is rank-3: scores s[t,n] = coords[t]·mu_n - |mu_n|^2/2 with
    coords = x @ Wp + (te @ Wp + bp).  coords is a K=4096 fp8-DoubleRow
    matmul with M=3; scores are then tiny K=4 matmuls producing s token-major
    [t, n] directly (no score transposes).  The per-token shift -|coords|^2/2
    is dropped (softmax over top-k and the top-k set are shift invariant).
  - top-3 selection via threshold = 3rd max (3 rounds of max + mask-out),
    gates g[t,n] = exp(s - max) * (s >= thr3) / sum(...)  (DVE chain).
  - z[t,:] (all 512 block-rank pairs) = x @ A_all, dense fp8 DoubleRow.
    zg = (z/8) * expand4(g) in fp8; delta = (8*block-diag(Bm)) matmul fp8;
    y = x + delta accumulated in bf16 SBUF and stored.
  - I/O is bf16 (host converts): x arrives pre-packed [128, 65536] bf16 in
    the exact SBUF tile order (fully contiguous DMA), y leaves the same way
    and is unpacked + upcast on host.  rel-err budget 2e-2 >> bf16 noise.
"""

import sys

if "/opt/trn_rl_repo" not in sys.path:
    sys.path.insert(0, "/opt/trn_rl_repo")

import numpy as np
import ml_dtypes

H = 4096
NB = 128
BLK = 32
R = 4
B = 4
S = 4096
NCORES = 8
TPC = (B * S) // NCORES  # tokens per core = 2048
T = 512                  # tokens per macrotile
NMT = TPC // T           # 4 macrotiles per core
NKT = H // 128           # 32 k-tiles over the hidden dim
BIG = 1.0e30
ZSC = 0.125              # z is scaled by 1/8 before fp8, Bm by 8

TRACE = False            # set by test.py for profiling runs
TRACE_DIR = None
LAST_RESULT = None       # BassKernelResults of the last run

_COMPILED = None


def _build():
    import concourse.bacc as bacc
    import concourse.tile as tile
    from concourse import mybir, masks

    f32 = mybir.dt.float32
    bf16 = mybir.dt.bfloat16
    f8 = mybir.dt.float8e4
    AF = mybir.ActivationFunctionType
    AL = mybir.AluOpType
    AX = mybir.AxisListType
    DR = mybir.MatmulPerfMode.DoubleRow

    nc = bacc.Bacc("TRN2", target_bir_lowering=False, debug=False,
                   num_devices=NCORES)

    xt_d = nc.dram_tensor("xt", [128, NMT * 8 * 2048], bf16,
                          kind="ExternalInput")
    xb_d = nc.dram_tensor("xb8", [128, NMT * 8 * 2048], f8,
                          kind="ExternalInput")
    az_d = nc.dram_tensor("az", [128, 4 * NKT * 128], f8, kind="ExternalInput")
    wp_d = nc.dram_tensor("wp", [128, NKT * 16], f8, kind="ExternalInput")
    bpk_d = nc.dram_tensor("bpk", [128, NKT * 128], f8, kind="ExternalInput")
    e_d = nc.dram_tensor("e", [128, 512], bf16, kind="ExternalInput")
    cen_d = nc.dram_tensor("cen", [4, 128], bf16, kind="ExternalInput")
    bias_d = nc.dram_tensor("bias", [128, 5], f32, kind="ExternalInput")
    yt_d = nc.dram_tensor("yt", [128, NMT * 8 * 2048], bf16,
                          kind="ExternalOutput")

    xt_ap = xt_d.ap()
    xb_ap = xb_d.ap()
    yt_ap = yt_d.ap()

    with tile.TileContext(nc) as tc:
        from contextlib import ExitStack
        with ExitStack() as ctx:
            cpool = ctx.enter_context(tc.tile_pool(name="consts", bufs=1))
            xpool = ctx.enter_context(tc.tile_pool(name="xg", bufs=16))
            xbpool = ctx.enter_context(tc.tile_pool(name="xb", bufs=16))
            zpool = ctx.enter_context(tc.tile_pool(name="zb", bufs=6))
            gpool = ctx.enter_context(tc.tile_pool(name="gate", bufs=3))
            spool = ctx.enter_context(tc.tile_pool(name="scal", bufs=4))
            pp = ctx.enter_context(tc.tile_pool(name="ps", bufs=2, space="PSUM"))

            # ---- persistent constants ----
            wp8 = cpool.tile([128, NKT * 16], f8, name="wp8", tag="wp8")
            nc.gpsimd.dma_start(wp8[:], wp_d.ap()[:])
            cen = cpool.tile([4, 128], bf16, name="cen", tag="cen")
            nc.gpsimd.dma_start(cen[:], cen_d.ap()[:])
            bias = cpool.tile([128, 5], f32, name="bias", tag="bias")
            nc.gpsimd.dma_start(bias[:], bias_d.ap()[:])
            az = []
            for q in range(4):
                t_az = cpool.tile([128, NKT * 128], f8, name=f"az{q}",
                                  tag=f"az{q}")
                nc.gpsimd.dma_start(
                    t_az[:], az_d.ap()[:, q * NKT * 128:(q + 1) * NKT * 128])
                az.append(t_az)
            esb = cpool.tile([128, 512], bf16, name="esb", tag="esb")
            nc.gpsimd.dma_start(esb[:], e_d.ap()[:])
            bpk = cpool.tile([128, NKT * 128], f8, name="bpk", tag="bpk")
            nc.gpsimd.dma_start(bpk[:], bpk_d.ap()[:])
            identf = cpool.tile([128, 128], f32, name="identf", tag="identf")
            masks.make_identity(nc, identf[:])

            NTS = T // 128  # token sub-tiles per macrotile

            for mt in range(NMT):
                base = mt * 8 * 2048
                # ---- fp8 x tiles first (gates the PE), then bf16 ----
                xbs = []
                for g in range(8):
                    xb = xbpool.tile([128, 4 * T], f8, name="xb", tag="xb")
                    eng = nc.scalar if (mt == 0 and g % 2 == 1) else nc.sync
                    eng.dma_start(xb[:],
                                  xb_ap[:, base + g * 2048:base + (g + 1) * 2048])
                    xbs.append(xb)
                xgs = []
                for g in range(8):
                    xg = xpool.tile([128, 4 * T], bf16, name="xg", tag="xg")
                    eng = nc.scalar if (mt == 0 and g % 2 == 1) else nc.sync
                    eng.dma_start(xg[:],
                                  xt_ap[:, base + g * 2048:base + (g + 1) * 2048])
                    xgs.append(xg)

                # ---- coords chain (fp8 DR, M=3) ----
                cp = pp.tile([16, T], f32, space="PSUM", name="cp", tag="cp",
                             bufs=1)
                for k2 in range(NKT // 2):
                    g, hl = k2 // 2, (k2 % 2) * 2
                    nc.tensor.matmul(
                        cp[:],
                        wp8[:, k2 * 32:(k2 + 1) * 32]
                        .rearrange("p (two m) -> p two m", two=2),
                        xbs[g][:, hl * T:(hl + 2) * T]
                        .rearrange("p (two t) -> p two t", two=2),
                        start=(k2 == 0), stop=(k2 == NKT // 2 - 1),
                        perf_mode=DR,
                    )
                csb = gpool.tile([4, T], bf16, name="csb", tag="csb", bufs=2)
                nc.gpsimd.memset(csb[:], 1.0)
                nc.scalar.activation(csb[0:3, :], cp[0:3, :], AF.Identity,
                                     bias=bias[0:3, 4:5], scale=1.0)

                # ---- scores (token-major) + DVE gate chain ----
                sp = pp.tile([128, 4 * 128], f32, space="PSUM", name="sp",
                             tag="sp", bufs=1)
                for ts in range(NTS):
                    nc.tensor.matmul(sp[:, ts * 128:(ts + 1) * 128],
                                     csb[:, ts * 128:(ts + 1) * 128],
                                     cen[:], start=True, stop=True)
                ggs = []
                for ts in range(NTS):
                    ssl = sp[:, ts * 128:(ts + 1) * 128]
                    m8 = spool.tile([128, 8], f32, name="m8", tag="m8")
                    nc.vector.max(m8[:], ssl)
                    nr1 = spool.tile([128, 1], f32, name="nr1", tag="nr1")
                    nc.vector.tensor_scalar_mul(nr1[:], m8[:, 0:1], -1.0)
                    ex = gpool.tile([128, 128], f32, name="ex", tag="ex")
                    nc.scalar.activation(ex[:], ssl, AF.Exp, bias=nr1[:],
                                         scale=1.0)
                    em = gpool.tile([128, 128], f32, name="em", tag="em")
                    zs = spool.tile([128, 1], f32, name="zs", tag="zs")
                    nc.vector.scalar_tensor_tensor(em[:], ssl, m8[:, 2:3],
                                                   ex[:], AL.is_ge, AL.mult,
                                                   accum_out=zs[:])
                    rz = spool.tile([128, 1], f32, name="rz", tag="rz")
                    nc.vector.reciprocal(rz[:], zs[:])
                    gg = gpool.tile([128, 128], f32, name="gg", tag="gg",
                                    bufs=NTS + 1)
                    nc.scalar.mul(gg[:], em[:], rz[:])
                    ggs.append(gg)

                # ---- z chunks (fp8 DoubleRow) ----
                zbs = []
                for q in range(4):
                    zp = pp.tile([128, T], f32, space="PSUM", name="zp",
                                 tag="zp")
                    for k2 in range(NKT // 2):
                        g, hl = k2 // 2, (k2 % 2) * 2
                        nc.tensor.matmul(
                            zp[:],
                            az[q][:, k2 * 256:(k2 + 1) * 256]
                            .rearrange("p (two m) -> p two m", two=2),
                            xbs[g][:, hl * T:(hl + 2) * T]
                            .rearrange("p (two t) -> p two t", two=2),
                            start=(k2 == 0), stop=(k2 == NKT // 2 - 1),
                            perf_mode=DR,
                        )
                    zb = zpool.tile([128, T], bf16, name="zb", tag="zb")
                    nc.scalar.activation(zb[:], zp[:], AF.Identity,
                                         bias=bias[:, q:q + 1], scale=ZSC)
                    zbs.append(zb)

                # ---- gates: transpose (PE, bf16) + expand matmuls ----
                gt_sb = gpool.tile([128, T], bf16, name="gt_sb", tag="gt_sb",
                                   bufs=2)
                g_ps = pp.tile([128, 4 * 128], f32, space="PSUM", name="g_ps",
                               tag="sp", bufs=1)
                for ts in range(NTS):
                    nc.tensor.transpose(g_ps[:, ts * 128:(ts + 1) * 128],
                                        ggs[ts][:], identf[:])
                nc.scalar.copy(gt_sb[:], g_ps[:])

                # ---- apply gates, delta, add into x, store ----
                for q in range(4):
                    gx = pp.tile([128, T], f32, space="PSUM", name="gx",
                                 tag="zp", bufs=2)
                    nc.tensor.matmul(gx[:],
                                     esb[:, q * 128:(q + 1) * 128],
                                     gt_sb[:],
                                     start=True, stop=True)
                    zg = zpool.tile([128, T], f8, name="zg", tag="zg")
                    nc.vector.tensor_mul(zg[:], zbs[q][:], gx[:])
                    for pair in range(4):
                        hc0 = q * 8 + pair * 2
                        g = hc0 // 4
                        dp = pp.tile([128, 2 * T], f32, space="PSUM", name="dp",
                                     tag="dp", bufs=2)
                        for h in range(2):
                            hc = hc0 + h
                            nc.tensor.matmul(dp[:, h * T:(h + 1) * T],
                                             bpk[:, hc * 128:(hc + 1) * 128],
                                             zg[:],
                                             start=True, stop=True)
                        xsl = xgs[g][:, (hc0 % 4) * T:(hc0 % 4 + 2) * T]
                        if (q * 4 + pair) % 2 == 1 and not (q == 3 and pair == 3):
                            nc.vector.tensor_add(xsl, xsl, dp[:])
                        else:
                            dsb = zpool.tile([128, 2 * T], bf16, name="dsb",
                                             tag="dsb", bufs=3)
                            nc.scalar.copy(dsb[:], dp[:])
                            nc.vector.tensor_add(xsl, xsl, dsb[:])
                        if hc0 % 4 == 2:
                            seng = nc.sync if (mt == NMT - 1 and g % 2 == 1) \
                                else nc.gpsimd
                            seng.dma_start(
                                yt_ap[:, base + g * 2048:base + (g + 1) * 2048],
                                xgs[g][:])

    nc.compile()
    return nc


def _prep_consts(task_emb, task_ids, Wp, bp, centers, A, Bm, adapter_scale):
    scale = float(np.asarray(adapter_scale))
    A_all = np.ascontiguousarray(
        A.transpose(1, 0, 2).reshape(H, NB * R).astype(np.float32))

    # az: [p, q, hc, m] = A_all[hc*128+p, q*128+m], fp8 e4m3 (DoubleRow pairs
    # of consecutive hc become the [K,2,M] interleave)
    az = (A_all.reshape(NKT, 128, 4, 128).transpose(1, 2, 0, 3)
          .reshape(128, 4 * NKT * 128).astype(ml_dtypes.float8_e4m3))
    az = np.ascontiguousarray(az)

    # wp8: [p, k2, two, c] = Wp[(2*k2+two)*128+p, c] (c padded 3->16:
    # fp8 DoubleRow LDWEIGHTS requires the pair stride to be 16B-aligned), fp8
    wpp = np.zeros((H, 16), np.float32)
    wpp[:, 0:3] = Wp.astype(np.float32)
    wp8 = (wpp.reshape(NKT // 2, 2, 128, 16)
           .transpose(2, 0, 1, 3).reshape(128, NKT * 16)
           .astype(ml_dtypes.float8_e4m3))
    wp8 = np.ascontiguousarray(wp8)

    # block-diag up-projection (x8 to keep fp8 in normal range; z is /8)
    bpk = np.zeros((128, NKT * 128), np.float32)
    for hc in range(NKT):
        for mblk in range(4):
            n = hc * 4 + mblk
            for r in range(R):
                row = (hc % 8) * 16 + mblk * 4 + r
                bpk[row, hc * 128 + mblk * 32: hc * 128 + mblk * 32 + 32] = \
                    Bm[n, r, :] * scale * 8.0
    bpk = bpk.astype(ml_dtypes.float8_e4m3)

    e_np = (np.arange(128)[:, None] == (np.arange(512)[None, :] // 4)) \
        .astype(ml_dtypes.bfloat16)

    # cen_aug: rows 0-2 = centers.T, row 3 = -|mu|^2/2
    cen = np.zeros((4, 128), np.float32)
    cen[0:3] = centers.T
    cen[3] = -0.5 * (centers ** 2).sum(-1)
    cen = np.ascontiguousarray(cen.astype(ml_dtypes.bfloat16))

    biases = []
    for c in range(NCORES):
        te = task_emb[int(np.asarray(task_ids)[c // 2])].astype(np.float32)
        b5 = np.zeros((128, 5), np.float32)
        zoff = (te @ A_all) * ZSC                                # [512]
        for q in range(4):
            b5[:, q] = zoff[q * 128:(q + 1) * 128]
        b5[0:3, 4] = te @ Wp + bp                                # coords bias
        biases.append(np.ascontiguousarray(b5))
    return az, wp8, bpk, e_np, cen, biases


def _pack_x(xc):
    # [TPC, H] f32 -> [128, NMT*8*2048] in kernel tile order (bf16 + fp8)
    t = np.ascontiguousarray(xc.reshape(NMT, T, 8, 4, 128)
                             .transpose(4, 0, 2, 3, 1)
                             .reshape(128, NMT * 8 * 2048))
    return t.astype(ml_dtypes.bfloat16), t.astype(ml_dtypes.float8_e4m3)


def _unpack_y(yt):
    # [128, NMT*8*2048] bf16 -> [TPC, H] f32
    t = yt.reshape(128, NMT, 8, 4, T).transpose(1, 4, 2, 3, 0)
    return t.reshape(TPC, H).astype(np.float32)


def kernel(x, task_ids, task_emb, Wp, bp, centers, A, Bm, adapter_scale):
    global _COMPILED, LAST_RESULT
    from concourse import bass_utils

    x = np.asarray(x, dtype=np.float32)
    task_ids = np.asarray(task_ids)
    task_emb = np.asarray(task_emb, dtype=np.float32)
    Wp = np.asarray(Wp, dtype=np.float32)
    bp = np.asarray(bp, dtype=np.float32)
    centers = np.asarray(centers, dtype=np.float32)
    A = np.asarray(A, dtype=np.float32)
    Bm = np.asarray(Bm, dtype=np.float32)

    if _COMPILED is None:
        _COMPILED = _build()
    nc = _COMPILED

    az, wp8, bpk, e_np, cen, biases = _prep_consts(
        task_emb, task_ids, Wp, bp, centers, A, Bm, adapter_scale)

    xf = x.reshape(B * S, H)
    in_maps = []
    for c in range(NCORES):
        xtc, xbc = _pack_x(xf[c * TPC:(c + 1) * TPC])
        in_maps.append({"xt": xtc, "xb8": xbc, "az": az, "wp": wp8,
                        "bpk": bpk, "e": e_np, "cen": cen,
                        "bias": biases[c]})

    kwargs = {}
    if TRACE:
        kwargs = dict(trace=True, tmpdir=TRACE_DIR)
    res = bass_utils.run_bass_kernel_spmd(
        nc, in_maps, core_ids=list(range(NCORES)), **kwargs)
    LAST_RESULT = res

    out = np.empty((B * S, H), np.float32)
    for c in range(NCORES):
        out[c * TPC:(c + 1) * TPC] = _unpack_y(res.results[c]["yt"])
    return out.reshape(B, S, H)
